# revision 26
# baseline (speedup 1.0000x reference)
"""Trainium2 Bass kernel for nn_ExoVariateEmbeddingMamba (v2: state-on-partitions).

Self-contained: accepts FULL (unsharded) inputs, shards the B*V=256 variate
sequences data-parallel across 8 NeuronCores (32 seqs/core), runs a Bass/Tile
kernel via run_bass_kernel_spmd, gathers the full [4, 64, 512] output.

v2 layout: the SSM inner loop iterates over channel-PAIRS (d, 64+d) with the
64 states on partitions: tile [s|s, L] (partitions 0-63 = states of channel d,
64-127 = states of channel 64+d). Per (seq n, pair k):
  dt_bc  = PE matmul(SELS_k, dt)         # rows k,64+k broadcast to halves
  dA     = ACT Exp(Apair[:,k] * dt_bc)   # fp32 decay factors
  dtx_bc = DMA row-pair broadcast from DRAM scratch (f16, batched 4 iters)
  u      = DVE  BB * dtx_bc              # f16, 2x mode
  h      = DVE scan(dA, u)               # 2 iters fused via dt-poison resets
  hc     = h * CC                        # f16; split DVE / GpSimd
  Y     += PE matmul(ONESS_k, hc)        # column-sum over states -> rows k,64+k
where BB/CC = [B^T;B^T], [C^T;C^T] are per-seq shared tiles (one PE matmul
each) and Y [d, L] accumulates in PSUM over the 64 pairs. Per-seq epilogue:
  P[:,n] = sum_t Y*G/L  (+ D * sum_t xc*G/L);  out = P^T @ (out_w@m_out_w)^T + b
All pre-SSM projections are rank<=4 in (xv, 1, shift(xv), step) as in v1.
"""
import numpy as np

import bass_rust
import concourse.tile as tile
from concourse import bacc, mybir
from concourse.bass_utils import run_bass_kernel_spmd

# problem shapes (hardcoded per contract)
B, L, V = 4, 1024, 64
D, S, R, DOUT = 128, 64, 8, 512
NCORES = 8
NSEQ = B * V // NCORES  # 32 sequences per core
HALF = L // 2

# f32 consts columns
APC = 0                 # Apair [128, 64]
W2L = APC + S           # W2T [128, 512]
DTB = W2L + DOUT        # dt_b col
DCL = DTB + 1           # D col
OBL = DCL + 1           # out_b rows 0..NSEQ-1 [*, 512]
NCOLF = OBL + DOUT

# bf16 consts columns
XP = 0                  # xprojT [128, 136] (cols 0..7 dt-rank, 8..135 B|C)
SBB = XP + R + 2 * S    # SELBB [128, 128]
SCC = SBB + D           # SELCC [128, 128]
LXC = SCC + D           # lhsT_xc rows 0..3, 128 cols
LZ = LXC + D            # lhsT_z rows 0..1, 128 cols
DTW = LZ + D            # dt_wT rows 0..7, 128 cols
SEL = DTW + D           # SELS [128, 64*128]
DIA = SEL + S * D       # diag(D) [128, 128]
NCOLB = DIA + D

_CACHE = {}
PROFILE = False
LAST_EXEC_NS = None
LAST_RESULTS = None
SIM_COMPAT = False      # CoreSim lacks Silu: emulate via Sigmoid + mult
BATCH = 4               # iters per broadcast DMA
BUFS_DA = 4
BUFS_UH = 4
BUFS_HC = 4
BUFS_BCB = 2
NF = 2                  # iters fused per scan/u/hc op (poisoned-dt resets)
HC_DVE_MOD = 7          # group % HC_DVE_MOD == 0 -> hc on DVE (rest GpSimd)
U_GPS_MOD = 0           # >0: group % U_GPS_MOD == 0 -> u on GpSimd
HC_SPLIT = 0            # 1: GpSimd hc as NF separate [D,L] ops
HC_PHASE = 0            # per-seq phase stagger of the DVE-hc groups

F32 = mybir.dt.float32
BF = mybir.dt.bfloat16
F16 = mybir.dt.float16
AF = mybir.ActivationFunctionType
OP = mybir.AluOpType


def _build(nseq: int):
    nc = bacc.Bacc("TRN2", target_bir_lowering=False, debug=False)
    consts_d = nc.dram_tensor("consts", [D, NCOLF], F32, kind="ExternalInput")
    constsb_d = nc.dram_tensor("constsb", [D, NCOLB], BF, kind="ExternalInput")
    constsh_d = nc.dram_tensor("constsh", [D, S * D], F16, kind="ExternalInput")
    xv4_d = nc.dram_tensor("xv4", [nseq, 4, L], BF, kind="ExternalInput")
    out_d = nc.dram_tensor("out", [nseq, DOUT], F32, kind="ExternalOutput")
    scr_d = nc.dram_tensor("scr", [nseq, 2, D, L], F16, kind="Internal")
    scr_t = scr_d.ap().tensor

    def silu(dst, src_psum, tmp_pool, tag):
        if SIM_COMPAT:
            tmp = tmp_pool.tile([D, L], F32, tag=tag + "_sg")
            nc.scalar.activation(out=tmp, in_=src_psum, func=AF.Sigmoid)
            nc.vector.tensor_tensor(out=dst, in0=tmp, in1=src_psum, op=OP.mult)
        else:
            nc.scalar.activation(out=dst, in_=src_psum, func=AF.Silu)

    with tile.TileContext(nc) as tc:
        with (
            tc.tile_pool(name="const", bufs=1) as constp,
            tc.tile_pool(name="pern", bufs=2) as pernp,
            tc.tile_pool(name="misc", bufs=1) as miscp,
            tc.tile_pool(name="bcb", bufs=BUFS_BCB) as bcbp,
            tc.tile_pool(name="da", bufs=BUFS_DA) as dap,
            tc.tile_pool(name="uh", bufs=BUFS_UH) as uhp,
            tc.tile_pool(name="hc", bufs=BUFS_HC) as hcp,
            tc.tile_pool(name="ps_pro", bufs=1, space="PSUM") as ps_pro,
            tc.tile_pool(name="ps_bc", bufs=2, space="PSUM") as ps_bc,
            tc.tile_pool(name="ps_y", bufs=1, space="PSUM") as ps_y,
        ):
            consts = constp.tile([D, NCOLF], F32)
            nc.sync.dma_start(out=consts, in_=consts_d.ap())
            constsb = constp.tile([D, NCOLB], BF)
            nc.sync.dma_start(out=constsb, in_=constsb_d.ap())
            constsh = constp.tile([D, S * D], F16)
            nc.sync.dma_start(out=constsh, in_=constsh_d.ap())
            P_mat = miscp.tile([D, nseq], F32)
            junk = miscp.tile([D, L], F32)

            for n in range(nseq):
                rhs4 = pernp.tile([4, L], BF, tag="rhs4")
                nc.sync.dma_start(out=rhs4, in_=xv4_d.ap()[n])

                # xc = Silu(conv proj), G = Silu(z proj)
                psum_xc = ps_pro.tile([D, L], F32, tag="pro")
                for c in range(2):
                    nc.tensor.matmul(
                        psum_xc[:, c * HALF:(c + 1) * HALF],
                        constsb[0:4, LXC:LXC + D],
                        rhs4[0:4, c * HALF:(c + 1) * HALF],
                        start=True, stop=True)
                xc = pernp.tile([D, L], BF, tag="xc")
                silu(xc, psum_xc, pernp, "xc")

                psum_z = ps_pro.tile([D, L], F32, tag="pro")
                for c in range(2):
                    nc.tensor.matmul(
                        psum_z[:, c * HALF:(c + 1) * HALF],
                        constsb[0:2, LZ:LZ + D],
                        rhs4[0:2, c * HALF:(c + 1) * HALF],
                        start=True, stop=True)
                G = pernp.tile([D, L], BF, tag="G")
                silu(G, psum_z, pernp, "G")

                # dtr [8, L] = xprojT[:, :8].T @ xc
                psum_dtr = ps_pro.tile([R, L], F32, tag="pro")
                for c in range(2):
                    nc.tensor.matmul(
                        psum_dtr[:, c * HALF:(c + 1) * HALF],
                        constsb[:, XP:XP + R],
                        xc[:, c * HALF:(c + 1) * HALF],
                        start=True, stop=True)
                dtr = pernp.tile([R, L], BF, tag="dtr")
                nc.scalar.copy(out=dtr, in_=psum_dtr)

                # dt = softplus(dt_wT.T @ dtr + dt_b)
                psum_dt = ps_pro.tile([D, L], F32, tag="pro")
                for c in range(2):
                    nc.tensor.matmul(
                        psum_dt[:, c * HALF:(c + 1) * HALF],
                        constsb[0:R, DTW:DTW + D],
                        dtr[0:R, c * HALF:(c + 1) * HALF],
                        start=True, stop=True)
                edt = pernp.tile([D, L], F32, tag="edt")
                nc.scalar.activation(out=edt, in_=psum_dt, func=AF.Exp,
                                     bias=consts[:, DTB:DTB + 1])
                dt_bf = pernp.tile([D, L], BF, tag="dtbf")
                nc.scalar.activation(out=dt_bf, in_=edt, func=AF.Ln, bias=1.0)

                # dtx = dt * xc -> f16, duplicated to DRAM scratch
                dtx = pernp.tile([D, L], F16, tag="dtx")
                nc.vector.tensor_tensor(out=dtx, in0=dt_bf, in1=xc, op=OP.mult)
                nc.sync.dma_start(out=scr_d.ap()[n, 0], in_=dtx)
                nc.sync.dma_start(out=scr_d.ap()[n, 1], in_=dtx)
                # poison dt[:, 0] = +1e9: every k-segment's first decay factor
                # becomes exp(A*1e9) = 0 (A < 0), so fused scans self-reset at
                # segment boundaries (h[0] = 0*prev + u[0]).
                nc.vector.memset(dt_bf[:, 0:1], 1e9)

                # BC [128, L] = xprojT[:, 8:136].T @ xc  (rows 0..63 B^T, 64..127 C^T)
                psum_bc0 = ps_pro.tile([D, L], F32, tag="pro")
                for c in range(2):
                    nc.tensor.matmul(
                        psum_bc0[:, c * HALF:(c + 1) * HALF],
                        constsb[:, XP + R:XP + R + 2 * S],
                        xc[:, c * HALF:(c + 1) * HALF],
                        start=True, stop=True)
                bc_sb = pernp.tile([D, L], BF, tag="bcsb")
                nc.scalar.copy(out=bc_sb, in_=psum_bc0)

                # BB = [B^T; B^T], CC = [C^T; C^T] (f16)
                psum_bb = ps_pro.tile([D, L], F32, tag="pro")
                for c in range(2):
                    nc.tensor.matmul(
                        psum_bb[:, c * HALF:(c + 1) * HALF],
                        constsb[:, SBB:SBB + D],
                        bc_sb[:, c * HALF:(c + 1) * HALF],
                        start=True, stop=True)
                BBt = pernp.tile([D, L], F16, tag="BB")
                nc.vector.tensor_copy(out=BBt, in_=psum_bb)
                psum_cc = ps_pro.tile([D, L], F32, tag="pro")
                for c in range(2):
                    nc.tensor.matmul(
                        psum_cc[:, c * HALF:(c + 1) * HALF],
                        constsb[:, SCC:SCC + D],
                        bc_sb[:, c * HALF:(c + 1) * HALF],
                        start=True, stop=True)
                CCt = pernp.tile([D, L], F16, tag="CC")
                nc.vector.tensor_copy(out=CCt, in_=psum_cc)

                Y = ps_y.tile([D, L], F32, tag="Y")

                for g in range(S // NF):
                    k0 = g * NF
                    if k0 % BATCH == 0:
                        bcb = bcbp.tile([D, BATCH * L], F16, tag="bcb")
                        ap = bass_rust.AP(
                            tensor=scr_t,
                            offset=n * 2 * D * L + k0 * L,
                            ap=[[192 * L, 2], [0, 64], [1, BATCH * L]])
                        nc.sync.dma_start(out=bcb, in_=ap)

                    # dt_bc per k: rows k, 64+k broadcast to partition halves;
                    # dA segments packed into one [D, NF*L] tile
                    dA = dap.tile([D, NF * L], F32, tag="dA")
                    for j in range(NF):
                        k = k0 + j
                        pbc = ps_bc.tile([D, L], F32, tag="bc")
                        for c in range(2):
                            nc.tensor.matmul(
                                pbc[:, c * HALF:(c + 1) * HALF],
                                constsb[:, SEL + k * D:SEL + (k + 1) * D],
                                dt_bf[:, c * HALF:(c + 1) * HALF],
                                start=True, stop=True)
                        nc.scalar.activation(
                            out=dA[:, j * L:(j + 1) * L], in_=pbc, func=AF.Exp,
                            scale=consts[:, APC + k:APC + k + 1])

                    u = uhp.tile([D, NF * L], F16, tag="u")
                    boff = (k0 % BATCH) * L
                    ueng = (nc.gpsimd if (U_GPS_MOD and g % U_GPS_MOD == 0)
                            else nc.vector)
                    ueng.tensor_tensor(
                        out=u[:, :].rearrange("p (a b) -> p a b", a=NF),
                        in0=BBt[:, :].unsqueeze(1).broadcast_to([D, NF, L]),
                        in1=bcb[:, boff:boff + NF * L].rearrange(
                            "p (a b) -> p a b", a=NF),
                        op=OP.mult)

                    # one fused scan; dt-poison zeroes dA at segment starts
                    h = uhp.tile([D, NF * L], F16, tag="h")
                    nc.vector.tensor_tensor_scan(
                        out=h, data0=dA, data1=u, initial=0.0,
                        op0=OP.mult, op1=OP.add)

                    hct = hcp.tile([D, NF * L], F16, tag="hc")
                    last = (g == S // NF - 1)
                    eng = (nc.vector
                           if ((g + n * HC_PHASE) % HC_DVE_MOD == 0 or last)
                           else nc.gpsimd)
                    if HC_SPLIT and eng is nc.gpsimd:
                        for j in range(NF):
                            eng.tensor_tensor(
                                out=hct[:, j * L:(j + 1) * L],
                                in0=h[:, j * L:(j + 1) * L],
                                in1=CCt, op=OP.mult)
                    else:
                        eng.tensor_tensor(
                            out=hct[:, :].rearrange("p (a b) -> p a b", a=NF),
                            in0=h[:, :].rearrange("p (a b) -> p a b", a=NF),
                            in1=CCt[:, :].unsqueeze(1).broadcast_to(
                                [D, NF, L]),
                            op=OP.mult)

                    for j in range(NF):
                        k = k0 + j
                        for c in range(2):
                            nc.tensor.matmul(
                                Y[:, c * HALF:(c + 1) * HALF],
                                constsh[:, k * D:(k + 1) * D],
                                hct[:, j * L + c * HALF:j * L + (c + 1) * HALF],
                                start=(k == 0), stop=False)

                # fold D-term: Y += diag(D) @ xc, then one readout STT
                for c in range(2):
                    nc.tensor.matmul(
                        Y[:, c * HALF:(c + 1) * HALF],
                        constsb[:, DIA:DIA + D],
                        xc[:, c * HALF:(c + 1) * HALF],
                        start=False, stop=True)
                nc.vector.scalar_tensor_tensor(
                    out=junk, in0=Y, scalar=1.0 / L, in1=G,
                    op0=OP.mult, op1=OP.mult,
                    accum_out=P_mat[:, n:n + 1])

            # out [nseq, DOUT] = P_mat.T @ W2T + out_b
            psum_out = ps_bc.tile([nseq, DOUT], F32, tag="bc")
            nc.tensor.matmul(psum_out, P_mat, consts[:, W2L:W2L + DOUT],
                             start=True, stop=True)
            out_sb = miscp.tile([nseq, DOUT], F32)
            nc.vector.tensor_tensor(out=out_sb, in0=psum_out,
                                    in1=consts[0:nseq, OBL:OBL + DOUT],
                                    op=OP.add)
            nc.sync.dma_start(out=out_d.ap(), in_=out_sb)

    nc.compile()
    return nc


def _host_prep(x_exo, in_w, in_b, m_in_w, conv_w, conv_b, xproj_w, dt_w, dt_b,
               A_log, D_in, m_out_w, out_w, out_b):
    import ml_dtypes
    f32 = np.float32
    w1 = (m_in_w @ in_w[:, 0]).astype(f32)
    b1 = (m_in_w @ in_b).astype(f32)
    w1x, w1z = w1[:D], w1[D:]
    b1x, b1z = b1[:D], b1[D:]
    cw0 = conv_w[:, 0, 0]
    cw1 = conv_w[:, 0, 1]
    a0 = (cw0 * w1x).astype(f32)
    a1 = (cw1 * w1x).astype(f32)
    c0 = (cw0 * b1x).astype(f32)
    cb = (cw1 * b1x + conv_b).astype(f32)
    A = (-np.exp(A_log)).astype(f32)          # [128, 64]
    W2 = (out_w @ m_out_w).astype(f32)

    consts = np.zeros((D, NCOLF), f32)
    # Apair col k: partitions 0..63 = A[k, :], 64..127 = A[64+k, :]
    for k in range(S):
        consts[0:S, APC + k] = A[k, :]
        consts[S:D, APC + k] = A[S + k, :]
    consts[:, W2L:W2L + DOUT] = W2.T
    consts[:, DTB] = dt_b
    consts[:, DCL] = D_in
    consts[0:NSEQ, OBL:OBL + DOUT] = np.tile(out_b, (NSEQ, 1))

    constsb = np.zeros((D, NCOLB), ml_dtypes.bfloat16)
    constsb[:, XP:XP + R + 2 * S] = xproj_w.T.astype(ml_dtypes.bfloat16)
    selbb = np.zeros((D, D), f32)
    selcc = np.zeros((D, D), f32)
    for p in range(D):
        selbb[p % S, p] = 1.0
        selcc[S + p % S, p] = 1.0
    constsb[:, SBB:SBB + D] = selbb
    constsb[:, SCC:SCC + D] = selcc
    constsb[0:4, LXC:LXC + D] = np.stack([a1, cb, a0, c0])
    constsb[0:2, LZ:LZ + D] = np.stack([w1z, b1z])
    constsb[0:R, DTW:DTW + D] = dt_w.T
    sels = np.zeros((D, S * D), f32)
    for k in range(S):
        sels[k, k * D:k * D + S] = 1.0
        sels[S + k, k * D + S:(k + 1) * D] = 1.0
    constsb[:, SEL:SEL + S * D] = sels
    constsb[:, DIA:DIA + D] = np.diag(D_in.astype(f32))

    constsh = np.zeros((D, S * D), np.float16)
    for k in range(S):
        constsh[0:S, k * D + k] = 1.0
        constsh[S:D, k * D + S + k] = 1.0

    xv_all = np.ascontiguousarray(
        x_exo.transpose(0, 2, 1).reshape(B * V, L)).astype(f32)
    xv4 = np.zeros((B * V, 4, L), f32)
    xv4[:, 0] = xv_all
    xv4[:, 1] = 1.0
    xv4[:, 2, 1:] = xv_all[:, :-1]
    xv4[:, 3, 1:] = 1.0
    xv4 = xv4.astype(ml_dtypes.bfloat16)
    return consts, constsb, constsh, xv4


def kernel(**inputs):
    consts, constsb, constsh, xv4 = _host_prep(
        inputs["x_exo"], inputs["in_w"], inputs["in_b"], inputs["m_in_w"],
        inputs["conv_w"], inputs["conv_b"], inputs["xproj_w"], inputs["dt_w"],
        inputs["dt_b"], inputs["A_log"], inputs["D"], inputs["m_out_w"],
        inputs["out_w"], inputs["out_b"])

    global LAST_EXEC_NS, LAST_RESULTS
    if _CACHE.get("nseq") != NSEQ:
        _CACHE["nc"] = _build(NSEQ)
        _CACHE["nseq"] = NSEQ
    nc = _CACHE["nc"]

    in_maps = []
    for c in range(NCORES):
        in_maps.append({
            "consts": consts,
            "constsb": constsb,
            "constsh": constsh,
            "xv4": np.ascontiguousarray(xv4[c * NSEQ:(c + 1) * NSEQ]),
        })
    res = run_bass_kernel_spmd(nc, in_maps, core_ids=list(range(NCORES)),
                               trace=PROFILE)
    LAST_EXEC_NS = res.exec_time_ns
    LAST_RESULTS = res
    out = np.concatenate([res.results[c]["out"] for c in range(NCORES)], axis=0)
    if out.shape[0] == B * V:
        out = out.reshape(B, V, DOUT)
    return out.astype(np.float32)
